# revision 35
# baseline (speedup 1.0000x reference)
"""APPNP model on 8 TRN2 NeuronCores.

Math (reference):
    h   = relu(X @ W1 + b1)          X: dense [N,F] from COO features
    z   = h @ W2 + b2                [N, L]
    p   = propagator @ z             propagator: [N, N]  (1 GiB f32)
    out = log_softmax(p, axis=1)

Distribution (8 cores): the propagator is row-sharded (core k owns rows
rk = [k*R, (k+1)*R)) and streamed through SBUF once — the memory-bound
part. The feature side (X, 16 MiB at fp8) is REPLICATED so every core
computes the full z [N, 16] locally; that removes the AllGather and with
it the runtime's ~50+us pre-collective global barrier, which otherwise
sits on the critical path. Phase 2 naturally emits z in [128-chunk, L]
orientation, which is exactly the stationary layout the propagation
matmul needs (contraction on partitions), so no transposes either.

Numerics: the propagation dominates and runs in fp8 e4m3 with DoubleRow
(2 contraction rows per PE cycle). The host pre-scales P by N so fp8
sees values in [0,1); the epilogue divides by N for free via the
activation `scale` input. W1 is pre-scaled by 32 (exact power of two) so
fp8 resolves its small values; the relu activation divides back via
scale=1/32. All accumulation stays f32 in PSUM; log-sum-exp runs in f32
(ones-matmul reduces over the L=16 partition rows; float32r streams at
full rate). Elementwise fp8/bf16 rounding averages out across the
16384-term dot products: measured end-to-end rel err ~3e-4.
"""

import sys

for _p in ("/opt/trn_rl_repo",):
    if _p not in sys.path:
        sys.path.append(_p)

import numpy as np

import concourse.bacc as bacc
import concourse.bass as bass
import concourse.mybir as mybir
from concourse import tile
from concourse.bass_utils import run_bass_kernel_spmd

N = 16384          # nodes
F = 1024           # features
H = 64             # hidden
L = 16             # labels
NC = 8             # cores
R = N // NC        # propagator rows per core = 2048

F32 = mybir.dt.float32
F32R = mybir.dt.float32r
BF16 = mybir.dt.bfloat16
FP8 = mybir.dt.float8e4

P_DT = FP8         # propagator (host pre-scales by N)
X_DT = FP8         # dense features (replicated; host pre-scales W1 by 32)
H_DT = BF16        # hidden activations
ZT_DT = FP8        # z stationary tiles (must match fp8 moving operand)
W1SCALE = 32.0     # exact power of two

P_BUFS = 12        # prefetch depth for 512 KiB propagator tiles
SEG = 2048         # node-column psum segment for the FC pipeline
XSEG = 4096        # node-column span of one 1 MiB X tile


def _build_nc(N=N, F=F, H=H, L=L, NC=NC, P_BUFS=P_BUFS, PSCALE=None):
    R = N // NC
    FJ = F // 256      # fp8 DoubleRow pair-groups over the feature dim
    ACH = N // 128     # 128-row z chunks
    NA2 = N // 256     # 256-row DoubleRow chunks of the propagation
    RB = R // 512      # 512-wide moving slices
    NSEG = N // SEG
    if PSCALE is None:
        PSCALE = float(N)
    nc = bacc.Bacc(None, target_bir_lowering=False, debug=False)

    pt = nc.dram_tensor("pt", [N, R], P_DT, kind="ExternalInput")  # P[rk,:].T * N
    # xtp[j, k, i, n] = X[n, j*256 + i*128 + k]  (DoubleRow pair layout)
    xtp = nc.dram_tensor("xtp", [FJ, 128, 2, N], X_DT, kind="ExternalInput")
    # w1p[k, j, i, h] = 32 * W1[j*256 + i*128 + k, h]
    w1p = nc.dram_tensor("w1p", [128, FJ, 2, H], X_DT, kind="ExternalInput")
    b1 = nc.dram_tensor("b1", [H, 1], F32, kind="ExternalInput")
    w2 = nc.dram_tensor("w2", [H, L], H_DT, kind="ExternalInput")
    b2r4 = nc.dram_tensor("b2r4", [128, 4, L], F32, kind="ExternalInput")
    onesc = nc.dram_tensor("onesc", [128, 1], F32R, kind="ExternalInput")
    onesr = nc.dram_tensor("onesr", [1, 128], F32R, kind="ExternalInput")
    out = nc.dram_tensor("out", [L, R], F32, kind="ExternalOutput")  # out^T

    with tile.TileContext(nc) as tc:
        with (
            tc.tile_pool(name="const", bufs=1) as const,
            tc.tile_pool(name="zpool", bufs=1) as zpool,
            tc.tile_pool(name="ppool", bufs=P_BUFS) as ppool,
        ):
            w1_s = const.tile([128, FJ, 2, H], X_DT)
            nc.scalar.dma_start(out=w1_s[:], in_=w1p[:])
            b1_s = const.tile([H, 1], F32)
            nc.scalar.dma_start(out=b1_s[:], in_=b1[:])
            w2_s = const.tile([H, L], H_DT)
            nc.scalar.dma_start(out=w2_s[:], in_=w2[:])
            b2r4_s = const.tile([128, 4, L], F32)
            nc.scalar.dma_start(out=b2r4_s[:], in_=b2r4[:])
            ones_col = const.tile([L, 1], F32R)
            nc.scalar.dma_start(out=ones_col[:], in_=onesc[:L, :])
            ones_row = const.tile([1, L], F32R)
            nc.scalar.dma_start(out=ones_row[:], in_=onesr[:, :L])

            # z for ALL nodes, chunked [128, ACH, L]: chunk a holds
            # z[a*128 + p, l] on partition p -- the propagation stationary
            zt_s = zpool.tile([128, ACH, L], ZT_DT)

            with (
                tc.tile_pool(name="xpool", bufs=8) as xpool,
                tc.tile_pool(name="hpool", bufs=3) as hpool,
                tc.tile_pool(name="ps1", bufs=1, space="PSUM") as ps1,
                nc.named_scope("fc"),
            ):
                # FC pipeline over node segments: fp8 DoubleRow X @ W1,
                # relu (undoes the x32 W1 scale), then z = h @ W2 + b2
                # emitted straight into zt_s chunks. X tiles ride the same
                # sync DMA FIFO as the propagator tiles, ahead of them, so
                # X gets full HBM bandwidth first (total DMA is serial
                # anyway) and the sequencer issues only 16 big X DMAs.
                for xs in range(N // XSEG):
                    xas = []
                    for j in range(FJ):
                        xa = xpool.tile([128, 2, XSEG], X_DT, tag="xa")
                        nc.sync.dma_start(
                            out=xa[:],
                            in_=xtp[j, :, :, xs * XSEG:(xs + 1) * XSEG])
                        xas.append(xa)
                    for sub in range(XSEG // SEG):
                        seg = xs * (XSEG // SEG) + sub
                        ph = ps1.tile([H, SEG], F32, tag="ph", bufs=1)
                        for j in range(FJ):
                            for nb in range(SEG // 512):
                                s0 = sub * SEG + nb * 512
                                nc.tensor.matmul(
                                    ph[:, nb * 512:(nb + 1) * 512],
                                    w1_s[:, j, :, :],
                                    xas[j][:, :, s0:s0 + 512],
                                    perf_mode=mybir.MatmulPerfMode.DoubleRow,
                                    start=(j == 0), stop=(j == FJ - 1),
                                )
                        h_seg = hpool.tile([H, SEG], H_DT, tag="hseg")
                        nc.scalar.activation(h_seg[:], ph[:],
                                             mybir.ActivationFunctionType.Relu,
                                             bias=b1_s[:], scale=1.0 / W1SCALE)
                        for q in range(SEG // 512):
                            pz4 = ps1.tile([128, 4, L], F32, tag="pz4", bufs=2)
                            for c in range(4):
                                col = q * 512 + c * 128
                                nc.tensor.matmul(pz4[:, c, :],
                                                 h_seg[:, col:col + 128],
                                                 w2_s[:])
                            g = seg * (SEG // 128) + q * 4
                            nc.vector.tensor_add(zt_s[:, g:g + 4, :], pz4[:],
                                                 b2r4_s[:])

            with (
                tc.tile_pool(name="epool", bufs=2) as epool,
                tc.tile_pool(name="ps2", bufs=1, space="PSUM") as ps2,
            ):
                # ---- propagation: out^T = z^T @ P^T, fp8 DoubleRow --------
                # k-tile pair (k, i) of chunk a2 maps to row (2*a2+i)*128+k,
                # i.e. stationary = two adjacent zt chunks.
                po = ps2.tile([L, R], F32)
                pt_r3 = pt.rearrange("(a k i) r -> a k i r", i=2, k=128)
                with nc.named_scope("prop"):
                    for a2 in range(NA2):
                        p_tile = ppool.tile([128, 2, R], P_DT, tag="p_tile")
                        nc.sync.dma_start(out=p_tile[:], in_=pt_r3[a2])
                        for rb in range(RB):
                            sl = slice(rb * 512, (rb + 1) * 512)
                            nc.tensor.matmul(
                                po[:, sl], zt_s[:, 2 * a2:2 * a2 + 2, :],
                                p_tile[:, :, sl],
                                perf_mode=mybir.MatmulPerfMode.DoubleRow,
                                start=(a2 == 0), stop=(a2 == NA2 - 1),
                            )

                # ---- log_softmax over the L=16 partition rows -------------
                # two column halves so psum scratch (tag aux) fits next to
                # po: 4 + 2*2 = 8 banks; activations grouped per function so
                # the ACT table loads once each.
                RH2 = R // 2
                with nc.named_scope("softmax"):
                    exps, sums, reps = [], [], []
                    for h2 in range(2):
                        co = slice(h2 * RH2, (h2 + 1) * RH2)
                        exp_s = epool.tile([L, RH2], F32R, tag="e",
                                           name=f"exp{h2}")
                        nc.scalar.activation(exp_s[:], po[:, co],
                                             mybir.ActivationFunctionType.Exp,
                                             scale=1.0 / PSCALE)
                        exps.append(exp_s)
                    for h2 in range(2):
                        sum_p = ps2.tile([L, RH2], F32, tag="aux", bufs=2)
                        for rb in range(max(1, RH2 // 512)):
                            sl = slice(rb * 512, min((rb + 1) * 512, RH2))
                            nc.tensor.matmul(sum_p[:1, sl], ones_col[:],
                                             exps[h2][:, sl])
                        sums.append(sum_p)
                    ls_s = epool.tile([1, R], F32R, tag="ls", bufs=1)
                    for h2 in range(2):
                        co = slice(h2 * RH2, (h2 + 1) * RH2)
                        nc.scalar.activation(ls_s[:, co], sums[h2][:1, :],
                                             mybir.ActivationFunctionType.Ln)
                    po_s = epool.tile([L, R], F32, tag="po_s", bufs=1)
                    nc.scalar.activation(po_s[:], po[:],
                                         mybir.ActivationFunctionType.Copy,
                                         scale=1.0 / PSCALE)
                    for h2 in range(2):
                        co = slice(h2 * RH2, (h2 + 1) * RH2)
                        rep_p = ps2.tile([L, RH2], F32, tag="aux", bufs=2)
                        for rb in range(max(1, RH2 // 512)):
                            sl = slice(rb * 512, min((rb + 1) * 512, RH2))
                            nc.tensor.matmul(rep_p[:, sl], ones_row[:],
                                             ls_s[:, co][:, sl])
                        reps.append(rep_p)
                    for h2 in range(2):
                        co = slice(h2 * RH2, (h2 + 1) * RH2)
                        fin_s = epool.tile([L, RH2], F32, tag="e")
                        nc.vector.tensor_sub(fin_s[:], po_s[:, co],
                                             reps[h2][:])
                        nc.sync.dma_start(out=out[:, co], in_=fin_s[:])

    nc.compile()
    return nc


_NC_CACHE = None


def _get_nc():
    global _NC_CACHE
    if _NC_CACHE is None:
        _NC_CACHE = _build_nc()
    return _NC_CACHE


def _densify(feature_indices, feature_values):
    rows = np.asarray(feature_indices[0]).astype(np.int64)
    cols = np.asarray(feature_indices[1]).astype(np.int64)
    vals = np.asarray(feature_values, dtype=np.float32)
    try:
        import scipy.sparse as sp
        X = np.asarray(
            sp.coo_matrix((vals, (rows, cols)), shape=(N, F)).todense(),
            dtype=np.float32)
    except ImportError:
        X = np.zeros((N, F), dtype=np.float32)
        np.add.at(X, (rows, cols), vals)
    return X


def make_in_maps(X, P, W1, b1, W2, b2, N=N, F=F, H=H, L=L, NC=NC):
    """Per-core input dicts from the full dense inputs (all float32)."""
    R = N // NC
    FJ = F // 256
    fp8 = mybir.dt.np(FP8)

    # xtp[j, k, i, n] = X[n, j*256 + i*128 + k]
    xtp = np.ascontiguousarray(
        np.asarray(X, np.float32).T.reshape(FJ, 2, 128, N).transpose(0, 2, 1, 3)
    ).astype(fp8)
    # w1p[k, j, i, h] = 32 * W1[j*256 + i*128 + k, h]
    w1p = np.ascontiguousarray(
        (np.asarray(W1, np.float32) * W1SCALE)
        .reshape(FJ, 2, 128, H).transpose(2, 0, 1, 3)).astype(fp8)
    b1c = np.ascontiguousarray(np.asarray(b1, np.float32).reshape(H, 1))
    W2h = np.asarray(W2, dtype=np.float32).astype(mybir.dt.np(H_DT))
    b2r4 = np.ascontiguousarray(
        np.tile(np.asarray(b2, np.float32).reshape(1, 1, L), (128, 4, 1)))
    ones128 = np.ones(128, dtype=np.float32)

    in_maps = []
    for k in range(NC):
        rk = slice(k * R, (k + 1) * R)
        # pair-interleave rows: [a2, k, i] -> row a2*256 + i*128 + k of P^T,
        # so partition k reads a contiguous 2-row (4 KB) run per tile
        pt_k = (P[rk, :].T * np.float32(N)).reshape(-1, 2, 128, R)
        pt_k = pt_k.transpose(0, 2, 1, 3).reshape(N, R)
        in_maps.append({
            "pt": np.ascontiguousarray(pt_k).astype(fp8),
            "xtp": xtp, "w1p": w1p, "b1": b1c, "w2": W2h, "b2r4": b2r4,
            "onesc": np.ascontiguousarray(ones128.reshape(128, 1)),
            "onesr": np.ascontiguousarray(ones128.reshape(1, 128)),
        })
    return in_maps


def kernel(feature_indices, feature_values, W1, b1, W2, b2, propagator):
    nc = _get_nc()

    X = _densify(feature_indices, feature_values)
    P = np.asarray(propagator, dtype=np.float32)
    in_maps = make_in_maps(X, P, W1, b1, W2, b2)

    res = run_bass_kernel_spmd(nc, in_maps, list(range(NC)))
    out_full = np.empty((N, L), dtype=np.float32)
    for k in range(NC):
        out_full[k * R:(k + 1) * R, :] = res.results[k]["out"].T
    return out_full


# revision 36
# speedup vs baseline: 1.1332x; 1.1332x over previous
"""APPNP model on 8 TRN2 NeuronCores.

Math (reference):
    h   = relu(X @ W1 + b1)          X: dense [N,F] from COO features
    z   = h @ W2 + b2                [N, L]
    p   = propagator @ z             propagator: [N, N]  (1 GiB f32)
    out = log_softmax(p, axis=1)

Distribution (8 cores): the propagator is row-sharded (core k owns rows
rk = [k*R, (k+1)*R)) and streamed through SBUF once — the memory-bound
part. The feature side (X, 16 MiB at fp8) is REPLICATED so every core
computes the full z [N, 16] locally; that removes the AllGather and with
it the runtime's ~50+us pre-collective global barrier, which otherwise
sits on the critical path. Phase 2 naturally emits z in [128-chunk, L]
orientation, which is exactly the stationary layout the propagation
matmul needs (contraction on partitions), so no transposes either.

Numerics: the propagation dominates and runs in fp8 e4m3 with DoubleRow
(2 contraction rows per PE cycle). The host pre-scales P by N so fp8
sees values in [0,1); the epilogue divides by N for free via the
activation `scale` input. W1 is pre-scaled by 32 (exact power of two) so
fp8 resolves its small values; the relu activation divides back via
scale=1/32. All accumulation stays f32 in PSUM; log-sum-exp runs in f32
(ones-matmul reduces over the L=16 partition rows; float32r streams at
full rate). Elementwise fp8/bf16 rounding averages out across the
16384-term dot products: measured end-to-end rel err ~3e-4.
"""

import sys

for _p in ("/opt/trn_rl_repo",):
    if _p not in sys.path:
        sys.path.append(_p)

import numpy as np

import concourse.bacc as bacc
import concourse.bass as bass
import concourse.mybir as mybir
from concourse import tile
from concourse.bass_utils import run_bass_kernel_spmd

N = 16384          # nodes
F = 1024           # features
H = 64             # hidden
L = 16             # labels
NC = 8             # cores
R = N // NC        # propagator rows per core = 2048

F32 = mybir.dt.float32
F32R = mybir.dt.float32r
BF16 = mybir.dt.bfloat16
FP8 = mybir.dt.float8e4

P_DT = FP8         # propagator (host pre-scales by N)
X_DT = FP8         # dense features (replicated; host pre-scales W1 by 32)
H_DT = BF16        # hidden activations
ZT_DT = FP8        # z stationary tiles (must match fp8 moving operand)
W1SCALE = 32.0     # exact power of two

P_BUFS = 12        # prefetch depth for 512 KiB propagator tiles
SEG = 2048         # node-column psum segment for the FC pipeline
XSEG = 4096        # node-column span of one 1 MiB X tile


def _build_nc(N=N, F=F, H=H, L=L, NC=NC, P_BUFS=P_BUFS, PSCALE=None):
    R = N // NC
    FJ = F // 256      # fp8 DoubleRow pair-groups over the feature dim
    ACH = N // 128     # 128-row z chunks
    NA2 = N // 256     # 256-row DoubleRow chunks of the propagation
    RB = R // 512      # 512-wide moving slices
    NSEG = N // SEG
    if PSCALE is None:
        PSCALE = float(N)
    nc = bacc.Bacc(None, target_bir_lowering=False, debug=False)

    pt = nc.dram_tensor("pt", [N, R], P_DT, kind="ExternalInput")  # P[rk,:].T * N
    # xtp[j, k, i, n] = X[n, j*256 + i*128 + k]  (DoubleRow pair layout)
    xtp = nc.dram_tensor("xtp", [FJ, 128, 2, N], X_DT, kind="ExternalInput")
    # w1p[k, j, i, h] = 32 * W1[j*256 + i*128 + k, h]
    w1p = nc.dram_tensor("w1p", [128, FJ, 2, H], X_DT, kind="ExternalInput")
    b1 = nc.dram_tensor("b1", [H, 1], F32, kind="ExternalInput")
    w2 = nc.dram_tensor("w2", [H, L], H_DT, kind="ExternalInput")
    b2r4 = nc.dram_tensor("b2r4", [128, 4, L], F32, kind="ExternalInput")
    onesc = nc.dram_tensor("onesc", [128, 1], F32R, kind="ExternalInput")
    onesr = nc.dram_tensor("onesr", [1, 128], F32R, kind="ExternalInput")
    out = nc.dram_tensor("out", [L, R], F32, kind="ExternalOutput")  # out^T

    with tile.TileContext(nc) as tc:
        with (
            tc.tile_pool(name="const", bufs=1) as const,
            tc.tile_pool(name="zpool", bufs=1) as zpool,
            tc.tile_pool(name="ppool", bufs=P_BUFS) as ppool,
        ):
            w1_s = const.tile([128, FJ, 2, H], X_DT)
            nc.scalar.dma_start(out=w1_s[:], in_=w1p[:])
            b1_s = const.tile([H, 1], F32)
            nc.scalar.dma_start(out=b1_s[:], in_=b1[:])
            w2_s = const.tile([H, L], H_DT)
            nc.scalar.dma_start(out=w2_s[:], in_=w2[:])
            b2r4_s = const.tile([128, 4, L], F32)
            nc.scalar.dma_start(out=b2r4_s[:], in_=b2r4[:])
            ones_col = const.tile([L, 1], F32R)
            nc.scalar.dma_start(out=ones_col[:], in_=onesc[:L, :])
            ones_row = const.tile([1, L], F32R)
            nc.scalar.dma_start(out=ones_row[:], in_=onesr[:, :L])

            # z for ALL nodes, chunked [128, ACH, L]: chunk a holds
            # z[a*128 + p, l] on partition p -- the propagation stationary
            zt_s = zpool.tile([128, ACH, L], ZT_DT)

            with (
                tc.tile_pool(name="xpool", bufs=8) as xpool,
                tc.tile_pool(name="hpool", bufs=3) as hpool,
                tc.tile_pool(name="ps1", bufs=1, space="PSUM") as ps1,
                nc.named_scope("fc"),
            ):
                # FC pipeline over node segments: fp8 DoubleRow X @ W1,
                # relu (undoes the x32 W1 scale), then z = h @ W2 + b2
                # emitted straight into zt_s chunks. X tiles ride the same
                # sync DMA FIFO as the propagator tiles, ahead of them, so
                # X gets full HBM bandwidth first (total DMA is serial
                # anyway) and the sequencer issues only 16 big X DMAs.
                for xs in range(N // XSEG):
                    xas = []
                    for j in range(FJ):
                        xa = xpool.tile([128, 2, XSEG], X_DT, tag="xa")
                        nc.sync.dma_start(
                            out=xa[:],
                            in_=xtp[j, :, :, xs * XSEG:(xs + 1) * XSEG])
                        xas.append(xa)
                    for sub in range(XSEG // SEG):
                        seg = xs * (XSEG // SEG) + sub
                        ph = ps1.tile([H, SEG], F32, tag="ph", bufs=1)
                        for j in range(FJ):
                            for nb in range(SEG // 512):
                                s0 = sub * SEG + nb * 512
                                nc.tensor.matmul(
                                    ph[:, nb * 512:(nb + 1) * 512],
                                    w1_s[:, j, :, :],
                                    xas[j][:, :, s0:s0 + 512],
                                    perf_mode=mybir.MatmulPerfMode.DoubleRow,
                                    start=(j == 0), stop=(j == FJ - 1),
                                )
                        h_seg = hpool.tile([H, SEG], H_DT, tag="hseg")
                        nc.scalar.activation(h_seg[:], ph[:],
                                             mybir.ActivationFunctionType.Relu,
                                             bias=b1_s[:], scale=1.0 / W1SCALE)
                        for q in range(SEG // 512):
                            pz4 = ps1.tile([128, 4, L], F32, tag="pz4", bufs=2)
                            for c in range(4):
                                col = q * 512 + c * 128
                                nc.tensor.matmul(pz4[:, c, :],
                                                 h_seg[:, col:col + 128],
                                                 w2_s[:])
                            g = seg * (SEG // 128) + q * 4
                            nc.vector.tensor_add(zt_s[:, g:g + 4, :], pz4[:],
                                                 b2r4_s[:])

            with (
                tc.tile_pool(name="epool", bufs=2) as epool,
                tc.tile_pool(name="ps2", bufs=1, space="PSUM") as ps2,
            ):
                # ---- propagation: out^T = z^T @ P^T, fp8 DoubleRow --------
                # k-tile pair (k, i) of chunk a2 maps to row (2*a2+i)*128+k,
                # i.e. stationary = two adjacent zt chunks.
                po = ps2.tile([L, R], F32)
                pt_r3 = pt.rearrange("(a i k) r -> a k i r", i=2, k=128)
                with nc.named_scope("prop"):
                    for a2 in range(NA2):
                        p_tile = ppool.tile([128, 2, R], P_DT, tag="p_tile")
                        nc.sync.dma_start(out=p_tile[:], in_=pt_r3[a2])
                        for rb in range(RB):
                            sl = slice(rb * 512, (rb + 1) * 512)
                            nc.tensor.matmul(
                                po[:, sl], zt_s[:, 2 * a2:2 * a2 + 2, :],
                                p_tile[:, :, sl],
                                perf_mode=mybir.MatmulPerfMode.DoubleRow,
                                start=(a2 == 0), stop=(a2 == NA2 - 1),
                            )

                # ---- log_softmax over the L=16 partition rows -------------
                # two column halves so psum scratch (tag aux) fits next to
                # po: 4 + 2*2 = 8 banks; activations grouped per function so
                # the ACT table loads once each.
                RH2 = R // 2
                with nc.named_scope("softmax"):
                    exps, sums, reps = [], [], []
                    for h2 in range(2):
                        co = slice(h2 * RH2, (h2 + 1) * RH2)
                        exp_s = epool.tile([L, RH2], F32R, tag="e",
                                           name=f"exp{h2}")
                        nc.scalar.activation(exp_s[:], po[:, co],
                                             mybir.ActivationFunctionType.Exp,
                                             scale=1.0 / PSCALE)
                        exps.append(exp_s)
                    for h2 in range(2):
                        sum_p = ps2.tile([L, RH2], F32, tag="aux", bufs=2)
                        for rb in range(max(1, RH2 // 512)):
                            sl = slice(rb * 512, min((rb + 1) * 512, RH2))
                            nc.tensor.matmul(sum_p[:1, sl], ones_col[:],
                                             exps[h2][:, sl])
                        sums.append(sum_p)
                    ls_s = epool.tile([1, R], F32R, tag="ls", bufs=1)
                    for h2 in range(2):
                        co = slice(h2 * RH2, (h2 + 1) * RH2)
                        nc.scalar.activation(ls_s[:, co], sums[h2][:1, :],
                                             mybir.ActivationFunctionType.Ln)
                    po_s = epool.tile([L, R], F32, tag="po_s", bufs=1)
                    nc.scalar.activation(po_s[:], po[:],
                                         mybir.ActivationFunctionType.Copy,
                                         scale=1.0 / PSCALE)
                    for h2 in range(2):
                        co = slice(h2 * RH2, (h2 + 1) * RH2)
                        rep_p = ps2.tile([L, RH2], F32, tag="aux", bufs=2)
                        for rb in range(max(1, RH2 // 512)):
                            sl = slice(rb * 512, min((rb + 1) * 512, RH2))
                            nc.tensor.matmul(rep_p[:, sl], ones_row[:],
                                             ls_s[:, co][:, sl])
                        reps.append(rep_p)
                    for h2 in range(2):
                        co = slice(h2 * RH2, (h2 + 1) * RH2)
                        fin_s = epool.tile([L, RH2], F32, tag="e")
                        nc.vector.tensor_sub(fin_s[:], po_s[:, co],
                                             reps[h2][:])
                        nc.sync.dma_start(out=out[:, co], in_=fin_s[:])

    nc.compile()
    return nc


_NC_CACHE = None


def _get_nc():
    global _NC_CACHE
    if _NC_CACHE is None:
        _NC_CACHE = _build_nc()
    return _NC_CACHE


def _densify(feature_indices, feature_values):
    rows = np.asarray(feature_indices[0]).astype(np.int64)
    cols = np.asarray(feature_indices[1]).astype(np.int64)
    vals = np.asarray(feature_values, dtype=np.float32)
    try:
        import scipy.sparse as sp
        X = np.asarray(
            sp.coo_matrix((vals, (rows, cols)), shape=(N, F)).todense(),
            dtype=np.float32)
    except ImportError:
        X = np.zeros((N, F), dtype=np.float32)
        np.add.at(X, (rows, cols), vals)
    return X


def make_in_maps(X, P, W1, b1, W2, b2, N=N, F=F, H=H, L=L, NC=NC):
    """Per-core input dicts from the full dense inputs (all float32)."""
    R = N // NC
    FJ = F // 256
    fp8 = mybir.dt.np(FP8)

    # xtp[j, k, i, n] = X[n, j*256 + i*128 + k]
    xtp = np.ascontiguousarray(
        np.asarray(X, np.float32).T.reshape(FJ, 2, 128, N).transpose(0, 2, 1, 3)
    ).astype(fp8)
    # w1p[k, j, i, h] = 32 * W1[j*256 + i*128 + k, h]
    w1p = np.ascontiguousarray(
        (np.asarray(W1, np.float32) * W1SCALE)
        .reshape(FJ, 2, 128, H).transpose(2, 0, 1, 3)).astype(fp8)
    b1c = np.ascontiguousarray(np.asarray(b1, np.float32).reshape(H, 1))
    W2h = np.asarray(W2, dtype=np.float32).astype(mybir.dt.np(H_DT))
    b2r4 = np.ascontiguousarray(
        np.tile(np.asarray(b2, np.float32).reshape(1, 1, L), (128, 4, 1)))
    ones128 = np.ones(128, dtype=np.float32)

    in_maps = []
    for k in range(NC):
        rk = slice(k * R, (k + 1) * R)
        pt_k = P[rk, :].T * np.float32(N)       # host pre-scale for fp8
        in_maps.append({
            "pt": np.ascontiguousarray(pt_k).astype(fp8),
            "xtp": xtp, "w1p": w1p, "b1": b1c, "w2": W2h, "b2r4": b2r4,
            "onesc": np.ascontiguousarray(ones128.reshape(128, 1)),
            "onesr": np.ascontiguousarray(ones128.reshape(1, 128)),
        })
    return in_maps


def kernel(feature_indices, feature_values, W1, b1, W2, b2, propagator):
    nc = _get_nc()

    X = _densify(feature_indices, feature_values)
    P = np.asarray(propagator, dtype=np.float32)
    in_maps = make_in_maps(X, P, W1, b1, W2, b2)

    res = run_bass_kernel_spmd(nc, in_maps, list(range(NC)))
    out_full = np.empty((N, L), dtype=np.float32)
    for k in range(NC):
        out_full[k * R:(k + 1) * R, :] = res.results[k]["out"].T
    return out_full


# revision 38
# speedup vs baseline: 1.1835x; 1.0444x over previous
"""APPNP model on 8 TRN2 NeuronCores.

Math (reference):
    h   = relu(X @ W1 + b1)          X: dense [N,F] from COO features
    z   = h @ W2 + b2                [N, L]
    p   = propagator @ z             propagator: [N, N]  (1 GiB f32)
    out = log_softmax(p, axis=1)

Distribution (8 cores): the propagator is row-sharded (core k owns rows
rk = [k*R, (k+1)*R)) and streamed through SBUF once — the memory-bound
part. The feature side (X, 16 MiB at fp8) is REPLICATED so every core
computes the full z [N, 16] locally; that removes the AllGather and with
it the runtime's ~50+us pre-collective global barrier, which otherwise
sits on the critical path. Phase 2 naturally emits z in [128-chunk, L]
orientation, which is exactly the stationary layout the propagation
matmul needs (contraction on partitions), so no transposes either.

Numerics: the propagation dominates and runs in fp8 e4m3 with DoubleRow
(2 contraction rows per PE cycle). The host pre-scales P by N so fp8
sees values in [0,1); the epilogue divides by N for free via the
activation `scale` input. W1 is pre-scaled by 32 (exact power of two) so
fp8 resolves its small values; the relu activation divides back via
scale=1/32. All accumulation stays f32 in PSUM; log-sum-exp runs in f32
(ones-matmul reduces over the L=16 partition rows; float32r streams at
full rate). Elementwise fp8/bf16 rounding averages out across the
16384-term dot products: measured end-to-end rel err ~3e-4.
"""

import sys

for _p in ("/opt/trn_rl_repo",):
    if _p not in sys.path:
        sys.path.append(_p)

import numpy as np

import concourse.bacc as bacc
import concourse.bass as bass
import concourse.mybir as mybir
from concourse import tile
from concourse.bass_utils import run_bass_kernel_spmd

N = 16384          # nodes
F = 1024           # features
H = 64             # hidden
L = 16             # labels
NC = 8             # cores
R = N // NC        # propagator rows per core = 2048

F32 = mybir.dt.float32
F32R = mybir.dt.float32r
BF16 = mybir.dt.bfloat16
FP8 = mybir.dt.float8e4

P_DT = FP8         # propagator (host pre-scales by N)
X_DT = FP8         # dense features (replicated; host pre-scales W1 by 32)
H_DT = BF16        # hidden activations
ZT_DT = FP8        # z stationary tiles (must match fp8 moving operand)
W1SCALE = 32.0     # exact power of two

P_BUFS = 12        # prefetch depth for 512 KiB propagator tiles
SEG = 2048         # node-column psum segment for the FC pipeline
XSEG = 4096        # node-column span of one 1 MiB X tile


def _build_nc(N=N, F=F, H=H, L=L, NC=NC, P_BUFS=P_BUFS, PSCALE=None):
    R = N // NC
    FJ = F // 256      # fp8 DoubleRow pair-groups over the feature dim
    ACH = N // 128     # 128-row z chunks
    NA2 = N // 256     # 256-row DoubleRow chunks of the propagation
    RB = R // 512      # 512-wide moving slices
    NSEG = N // SEG
    if PSCALE is None:
        PSCALE = float(N)
    nc = bacc.Bacc(None, target_bir_lowering=False, debug=False)

    pt = nc.dram_tensor("pt", [N, R], P_DT, kind="ExternalInput")  # P[rk,:].T * N
    # xtp[j, k, i, n] = X[n, j*256 + i*128 + k]  (DoubleRow pair layout)
    xtp = nc.dram_tensor("xtp", [FJ, 128, 2, N], X_DT, kind="ExternalInput")
    # w1p[k, j, i, h] = 32 * W1[j*256 + i*128 + k, h]
    w1p = nc.dram_tensor("w1p", [128, FJ, 2, H], X_DT, kind="ExternalInput")
    b1 = nc.dram_tensor("b1", [H, 1], F32, kind="ExternalInput")
    w2 = nc.dram_tensor("w2", [H, L], H_DT, kind="ExternalInput")
    b2r4 = nc.dram_tensor("b2r4", [128, 4, L], F32, kind="ExternalInput")
    onesc = nc.dram_tensor("onesc", [128, 1], F32R, kind="ExternalInput")
    onesr = nc.dram_tensor("onesr", [1, 128], F32R, kind="ExternalInput")
    out = nc.dram_tensor("out", [L, R], F32, kind="ExternalOutput")  # out^T

    with tile.TileContext(nc) as tc:
        with (
            tc.tile_pool(name="const", bufs=1) as const,
            tc.tile_pool(name="zpool", bufs=1) as zpool,
            tc.tile_pool(name="ppool", bufs=P_BUFS) as ppool,
        ):
            w1_s = const.tile([128, FJ, 2, H], X_DT)
            nc.scalar.dma_start(out=w1_s[:], in_=w1p[:])
            b1_s = const.tile([H, 1], F32)
            nc.scalar.dma_start(out=b1_s[:], in_=b1[:])
            w2_s = const.tile([H, L], H_DT)
            nc.scalar.dma_start(out=w2_s[:], in_=w2[:])
            b2r4_s = const.tile([128, 4, L], F32)
            nc.scalar.dma_start(out=b2r4_s[:], in_=b2r4[:])
            ones_col = const.tile([L, 1], F32R)
            nc.scalar.dma_start(out=ones_col[:], in_=onesc[:L, :])
            ones_row = const.tile([1, L], F32R)
            nc.scalar.dma_start(out=ones_row[:], in_=onesr[:, :L])

            # z for ALL nodes, chunked [128, ACH, L]: chunk a holds
            # z[a*128 + p, l] on partition p -- the propagation stationary
            zt_s = zpool.tile([128, ACH, L], ZT_DT)

            with (
                tc.tile_pool(name="xpool", bufs=8) as xpool,
                tc.tile_pool(name="hpool", bufs=3) as hpool,
                tc.tile_pool(name="ps1", bufs=1, space="PSUM") as ps1,
                nc.named_scope("fc"),
            ):
                # FC pipeline over node segments: fp8 DoubleRow X @ W1,
                # relu (undoes the x32 W1 scale), then z = h @ W2 + b2
                # emitted straight into zt_s chunks. X tiles ride the same
                # sync DMA FIFO as the propagator tiles, ahead of them, so
                # X gets full HBM bandwidth first (total DMA is serial
                # anyway) and the sequencer issues only 16 big X DMAs.
                for xs in range(N // XSEG):
                    xas = []
                    for j in range(FJ):
                        xa = xpool.tile([128, 2, XSEG], X_DT, tag="xa")
                        nc.sync.dma_start(
                            out=xa[:],
                            in_=xtp[j, :, :, xs * XSEG:(xs + 1) * XSEG])
                        xas.append(xa)
                    for sub in range(XSEG // SEG):
                        seg = xs * (XSEG // SEG) + sub
                        ph = ps1.tile([H, SEG], F32, tag="ph", bufs=1)
                        for j in range(FJ):
                            for nb in range(SEG // 512):
                                s0 = sub * SEG + nb * 512
                                nc.tensor.matmul(
                                    ph[:, nb * 512:(nb + 1) * 512],
                                    w1_s[:, j, :, :],
                                    xas[j][:, :, s0:s0 + 512],
                                    perf_mode=mybir.MatmulPerfMode.DoubleRow,
                                    start=(j == 0), stop=(j == FJ - 1),
                                )
                        h_seg = hpool.tile([H, SEG], H_DT, tag="hseg")
                        nc.scalar.activation(h_seg[:], ph[:],
                                             mybir.ActivationFunctionType.Relu,
                                             bias=b1_s[:], scale=1.0 / W1SCALE)
                        for q in range(SEG // 512):
                            pz4 = ps1.tile([128, 4, L], F32, tag="pz4", bufs=2)
                            for c in range(4):
                                col = q * 512 + c * 128
                                nc.tensor.matmul(pz4[:, c, :],
                                                 h_seg[:, col:col + 128],
                                                 w2_s[:])
                            g = seg * (SEG // 128) + q * 4
                            nc.vector.tensor_add(zt_s[:, g:g + 4, :], pz4[:],
                                                 b2r4_s[:])

            with (
                tc.tile_pool(name="epool", bufs=2) as epool,
                tc.tile_pool(name="ps2", bufs=1, space="PSUM") as ps2,
            ):
                # ---- propagation: out^T = z^T @ P^T, fp8 DoubleRow --------
                # k-tile pair (k, i) of chunk a2 maps to row (2*a2+i)*128+k,
                # i.e. stationary = two adjacent zt chunks.
                po = ps2.tile([L, R], F32)
                pt_r3 = pt.rearrange("(a i k) r -> a k i r", i=2, k=128)
                with nc.named_scope("prop"):
                    for a2 in range(NA2):
                        p_tile = ppool.tile([128, 2, R], P_DT, tag="p_tile")
                        nc.sync.dma_start(out=p_tile[:], in_=pt_r3[a2])
                        for rb in range(RB):
                            sl = slice(rb * 512, (rb + 1) * 512)
                            nc.tensor.matmul(
                                po[:, sl], zt_s[:, 2 * a2:2 * a2 + 2, :],
                                p_tile[:, :, sl],
                                perf_mode=mybir.MatmulPerfMode.DoubleRow,
                                start=(a2 == 0), stop=(a2 == NA2 - 1),
                            )

                # ---- log_softmax over the L=16 partition rows -------------
                # two column halves so psum scratch (tag aux) fits next to
                # po: 4 + 2*2 = 8 banks; activations grouped per function so
                # the ACT table loads once each.
                RH2 = R // 2
                with nc.named_scope("softmax"):
                    exps, sums, reps = [], [], []
                    for h2 in range(2):
                        co = slice(h2 * RH2, (h2 + 1) * RH2)
                        exp_s = epool.tile([L, RH2], F32R, tag="e",
                                           name=f"exp{h2}")
                        nc.scalar.activation(exp_s[:], po[:, co],
                                             mybir.ActivationFunctionType.Exp,
                                             scale=1.0 / PSCALE)
                        exps.append(exp_s)
                    for h2 in range(2):
                        sum_p = ps2.tile([L, RH2], F32, tag="aux", bufs=2)
                        for rb in range(max(1, RH2 // 512)):
                            sl = slice(rb * 512, min((rb + 1) * 512, RH2))
                            nc.tensor.matmul(sum_p[:1, sl], ones_col[:],
                                             exps[h2][:, sl])
                        sums.append(sum_p)
                    ls_s = epool.tile([1, R], F32R, tag="ls", bufs=1)
                    for h2 in range(2):
                        co = slice(h2 * RH2, (h2 + 1) * RH2)
                        nc.scalar.activation(ls_s[:, co], sums[h2][:1, :],
                                             mybir.ActivationFunctionType.Ln)
                    po_s = epool.tile([L, R], F32, tag="po_s", bufs=1)
                    nc.scalar.activation(po_s[:], po[:],
                                         mybir.ActivationFunctionType.Copy,
                                         scale=1.0 / PSCALE)
                    for h2 in range(2):
                        co = slice(h2 * RH2, (h2 + 1) * RH2)
                        rep_p = ps2.tile([L, RH2], F32, tag="aux", bufs=2)
                        for rb in range(max(1, RH2 // 512)):
                            sl = slice(rb * 512, min((rb + 1) * 512, RH2))
                            nc.tensor.matmul(rep_p[:, sl], ones_row[:],
                                             ls_s[:, co][:, sl])
                        reps.append(rep_p)
                    for h2 in range(2):
                        co = slice(h2 * RH2, (h2 + 1) * RH2)
                        fin_s = epool.tile([L, RH2], F32, tag="e")
                        nc.vector.tensor_sub(fin_s[:], po_s[:, co],
                                             reps[h2][:])
                        nc.sync.dma_start(out=out[:, co], in_=fin_s[:])

    nc.compile()
    return nc


_NC_CACHE = None


def _get_nc():
    global _NC_CACHE
    if _NC_CACHE is None:
        _NC_CACHE = _build_nc()
    return _NC_CACHE


def _build_warmup_nc():
    """Tiny SPMD kernel: first-exec of a fresh big NEFF occasionally hits
    NRT_EXEC_UNIT_UNRECOVERABLE; running a trivial NEFF first absorbs it."""
    nc = bacc.Bacc(None, target_bir_lowering=False, debug=False)
    x = nc.dram_tensor("x", [128, 128], F32, kind="ExternalInput")
    y = nc.dram_tensor("y", [128, 128], F32, kind="ExternalOutput")
    with tile.TileContext(nc) as tc:
        with tc.tile_pool(name="sbuf", bufs=1) as pool:
            t = pool.tile([128, 128], F32)
            nc.sync.dma_start(out=t[:], in_=x[:])
            nc.scalar.mul(out=t[:], in_=t[:], mul=1.0)
            nc.sync.dma_start(out=y[:], in_=t[:])
    nc.compile()
    return nc


_WARM_CACHE = None


def _run_warmup():
    global _WARM_CACHE
    try:
        if _WARM_CACHE is None:
            _WARM_CACHE = _build_warmup_nc()
        xs = np.zeros((128, 128), dtype=np.float32)
        run_bass_kernel_spmd(_WARM_CACHE, [{"x": xs}] * NC, list(range(NC)))
    except Exception:
        pass


def _densify(feature_indices, feature_values):
    rows = np.asarray(feature_indices[0]).astype(np.int64)
    cols = np.asarray(feature_indices[1]).astype(np.int64)
    vals = np.asarray(feature_values, dtype=np.float32)
    try:
        import scipy.sparse as sp
        X = np.asarray(
            sp.coo_matrix((vals, (rows, cols)), shape=(N, F)).todense(),
            dtype=np.float32)
    except ImportError:
        X = np.zeros((N, F), dtype=np.float32)
        np.add.at(X, (rows, cols), vals)
    return X


def make_in_maps(X, P, W1, b1, W2, b2, N=N, F=F, H=H, L=L, NC=NC):
    """Per-core input dicts from the full dense inputs (all float32)."""
    R = N // NC
    FJ = F // 256
    fp8 = mybir.dt.np(FP8)

    # xtp[j, k, i, n] = X[n, j*256 + i*128 + k]
    xtp = np.ascontiguousarray(
        np.asarray(X, np.float32).T.reshape(FJ, 2, 128, N).transpose(0, 2, 1, 3)
    ).astype(fp8)
    # w1p[k, j, i, h] = 32 * W1[j*256 + i*128 + k, h]
    w1p = np.ascontiguousarray(
        (np.asarray(W1, np.float32) * W1SCALE)
        .reshape(FJ, 2, 128, H).transpose(2, 0, 1, 3)).astype(fp8)
    b1c = np.ascontiguousarray(np.asarray(b1, np.float32).reshape(H, 1))
    W2h = np.asarray(W2, dtype=np.float32).astype(mybir.dt.np(H_DT))
    b2r4 = np.ascontiguousarray(
        np.tile(np.asarray(b2, np.float32).reshape(1, 1, L), (128, 4, 1)))
    ones128 = np.ones(128, dtype=np.float32)

    in_maps = []
    for k in range(NC):
        rk = slice(k * R, (k + 1) * R)
        pt_k = P[rk, :].T * np.float32(N)       # host pre-scale for fp8
        in_maps.append({
            "pt": np.ascontiguousarray(pt_k).astype(fp8),
            "xtp": xtp, "w1p": w1p, "b1": b1c, "w2": W2h, "b2r4": b2r4,
            "onesc": np.ascontiguousarray(ones128.reshape(128, 1)),
            "onesr": np.ascontiguousarray(ones128.reshape(1, 128)),
        })
    return in_maps


def kernel(feature_indices, feature_values, W1, b1, W2, b2, propagator):
    nc = _get_nc()

    X = _densify(feature_indices, feature_values)
    P = np.asarray(propagator, dtype=np.float32)
    in_maps = make_in_maps(X, P, W1, b1, W2, b2)

    _run_warmup()
    res = None
    for attempt in range(3):
        try:
            res = run_bass_kernel_spmd(nc, in_maps, list(range(NC)))
            break
        except Exception:
            if attempt == 2:
                raise
            import time
            time.sleep(10)
            _run_warmup()
    out_full = np.empty((N, L), dtype=np.float32)
    for k in range(NC):
        out_full[k * R:(k + 1) * R, :] = res.results[k]["out"].T
    return out_full
